# revision 2
# baseline (speedup 1.0000x reference)
"""Trainium2 Bass kernel for nn_DRAM_MAC_temporal_encoding (polynomial attention).

Math (QK_mul=1):
    out = sum_i coef_i * (x @ (y-OFF)^i) * decay
        = (x * decay) @ P(y-OFF)            # P = Horner cubic, elementwise
so the whole problem is ONE [S,64]@[64,S] matmul per (b,h) head plus the
384 MiB output write -> memory-bound. The tiny elementwise prep (poly on y,
row-scaling x, transposes, bf16 hi/lo split) runs on host; the device does
matmuls + store.

QK_mul=0: out = sum_i coef_i * ((x*d^i) @ (y-OFF)^i)
        = concat_i(x*d^i) @ concat_i(coef_i*(y-OFF)^i)   # K=256, same kernel
          with 4 K-chunks instead of 1.

Precision/speed: fp32 matmuls are 4 cycles/row and fp32r is ~500x off fp32
accuracy here, so each K=64 product is computed as an exact-ish 4-term bf16
hi/lo split needing only 2 K=128 bf16 matmuls and no duplicated data:
    A@W ~= [A_hi;A_lo]@[W_hi;W_lo] + [A_lo;A_hi]@[W_hi;W_lo]
(covers A_hi@W_hi + A_lo@W_lo + A_lo@W_hi + A_hi@W_lo; measured absmax err
~6.5e-4 on out scale ~538, ~4x the fp32 envelope.)

Sharding: 24 (b,h) heads -> 3 per core across 8 cores.
"""

import ml_dtypes
import numpy as np

import concourse.mybir as mybir
import concourse.tile as tile
from concourse import bacc
from concourse.bass_utils import run_bass_kernel_spmd

C = [0.17393044, 0.15653739, 0.14088365, 0.12679529, 5.51975209,
     4.96777688, 4.4709992, -1.44776001, -1.30298401, 46.05483778]
MAX_ORDER = 3
X_MAX = 0.9
OFFSET = 0.45

B, H, S, D = 2, 12, 2048, 64
BH = B * H
N_CORES = 8
BLK = BH // N_CORES  # heads per core

M_TILE = 128   # output rows per matmul (PSUM partitions)
N_TILE = 512   # output cols per matmul (one fp32 PSUM bank)
KP = 128       # stacked [hi;lo] contraction rows per chunk

BF16 = ml_dtypes.bfloat16

_NC_CACHE = {}


def _coefs():
    cs = []
    idx = 0
    for i in range(MAX_ORDER + 1):
        n_j = MAX_ORDER - i + 1
        cs.append(sum(C[idx + j] * X_MAX ** j for j in range(n_j)))
        idx += n_j
    return cs  # [c0, c1, c2, c3]


def _build_nc(kcs):
    """Device kernel: per core, BLK independent [S,S] output blocks, each
    sum over kcs K-chunks of 2 bf16 matmuls (hi/lo 4-term split)."""
    nc = bacc.Bacc(None, target_bir_lowering=False)
    at_d = nc.dram_tensor("at", [BLK, kcs, KP, S], mybir.dt.bfloat16,
                          kind="ExternalInput")
    w_d = nc.dram_tensor("w", [BLK, kcs, KP, S], mybir.dt.bfloat16,
                         kind="ExternalInput")
    out_d = nc.dram_tensor("out", [BLK, S, S], mybir.dt.float32,
                           kind="ExternalOutput")

    # 8 staging buffers absorb DMA completion jitter (measured ~2x better
    # than 4 on HW; 12 regresses). The kcs=4 path drops to 4 to fit SBUF.
    out_bufs = 8 if kcs == 1 else 4
    # One DVE copy drains all 4 PSUM banks of a row-block (P6: per-op DRAIN
    # overhead makes op count, not bytes, the DVE cost driver).
    FUSE = 4
    with tile.TileContext(nc) as tc:
        with (
            tc.tile_pool(name="inp", bufs=1) as inp,
            tc.tile_pool(name="ps", bufs=8 // FUSE, space="PSUM") as psp,
            tc.tile_pool(name="outp", bufs=out_bufs) as outp,
        ):
            # Prefetch every input tile up front so the steady-state DMA
            # queues carry only output stores. The [lo;hi] variant (v=1) is a
            # row-swap of the stored [hi;lo] data, materialized by two
            # half-tile DMAs from the same DRAM buffer instead of duplicating
            # the upload + HBM read.
            at_ts, w_ts = {}, {}
            half = KP // 2
            for blk in range(BLK):
                for c in range(kcs):
                    ta = inp.tile([KP, S], mybir.dt.bfloat16,
                                  tag=f"at{blk}_{c}_0")
                    nc.sync.dma_start(ta[:], at_d[blk, c])
                    at_ts[(blk, c, 0)] = ta
                    t = inp.tile([KP, S], mybir.dt.bfloat16,
                                 tag=f"at{blk}_{c}_1")
                    nc.sync.dma_start(t[:half], ta[half:])
                    nc.sync.dma_start(t[half:], ta[:half])
                    at_ts[(blk, c, 1)] = t
                    t = inp.tile([KP, S], mybir.dt.bfloat16,
                                 tag=f"w{blk}_{c}")
                    nc.sync.dma_start(t[:], w_d[blk, c])
                    w_ts[(blk, c)] = t
            n_mm = 2 * kcs
            for blk in range(BLK):
                for st in range(S // M_TILE):
                    ot = outp.tile([M_TILE, S], mybir.dt.float32, tag="ot")
                    for g in range(S // (N_TILE * FUSE)):
                        ps = psp.tile([M_TILE, FUSE * N_TILE],
                                      mybir.dt.float32, tag="ps")
                        for f in range(FUSE):
                            nt = g * FUSE + f
                            i = 0
                            for c in range(kcs):
                                for v in range(2):
                                    nc.tensor.matmul(
                                        ps[:, f * N_TILE:(f + 1) * N_TILE],
                                        at_ts[(blk, c, v)][
                                            :, st * M_TILE:(st + 1) * M_TILE],
                                        w_ts[(blk, c)][
                                            :, nt * N_TILE:(nt + 1) * N_TILE],
                                        start=(i == 0),
                                        stop=(i == n_mm - 1),
                                    )
                                    i += 1
                        nc.vector.tensor_copy(
                            ot[:, g * FUSE * N_TILE:(g + 1) * FUSE * N_TILE],
                            ps[:])
                    nc.sync.dma_start(
                        out_d[blk, st * M_TILE:(st + 1) * M_TILE, :], ot[:])
    nc.compile()
    return nc


def _get_nc(kcs):
    if kcs not in _NC_CACHE:
        _NC_CACHE[kcs] = _build_nc(kcs)
    return _NC_CACHE[kcs]


def _hilo(a):
    """Split f32 array -> (hi, lo) bf16 pair with hi+lo ~= a (16-17 mant bits)."""
    hi = a.astype(BF16)
    lo = (a - hi.astype(np.float32)).astype(BF16)
    return hi, lo


def _prepare(x, y, dm, qk):
    """Host prep -> (at, w) bf16 arrays, both [BH, kcs, KP, S] with [hi;lo]
    rows; the [lo;hi] variant needed for the cross terms is formed on device
    by a half-swapped DMA of the same data."""
    c0, c1, c2, c3 = _coefs()
    yo = y - OFFSET  # [B,H,D,S]
    if qk:
        a_t = [(x * dm[None, None, :, :]).transpose(0, 1, 3, 2)]  # [B,H,D,S]
        ws = [((c3 * yo + c2) * yo + c1) * yo + c0]
        kcs = 1
    else:
        d = dm[:, 0]
        a_t, ws = [], []
        di = np.ones_like(d)
        yi = np.ones_like(yo)
        for ci in (c0, c1, c2, c3):
            a_t.append((x * di[None, None, :, None]).transpose(0, 1, 3, 2))
            ws.append(ci * yi)
            di = di * d
            yi = yi * yo
        kcs = 4
    at = np.empty((BH, kcs, KP, S), dtype=BF16)
    w = np.empty((BH, kcs, KP, S), dtype=BF16)
    for c in range(kcs):
        a_hi, a_lo = _hilo(np.ascontiguousarray(a_t[c], np.float32)
                           .reshape(BH, D, S))
        w_hi, w_lo = _hilo(np.ascontiguousarray(ws[c], np.float32)
                           .reshape(BH, D, S))
        at[:, c, :D] = a_hi
        at[:, c, D:] = a_lo
        w[:, c, :D] = w_hi
        w[:, c, D:] = w_lo
    return at, w, kcs


def kernel(**inputs):
    x = np.asarray(inputs["x"], dtype=np.float32)
    y = np.asarray(inputs["y"], dtype=np.float32)
    dm = np.asarray(inputs["decay_mask"], dtype=np.float32)
    qk = int(np.asarray(inputs["QK_mul"]))

    at, w, kcs = _prepare(x, y, dm, qk)
    nc = _get_nc(kcs)

    in_maps = [
        {"at": at[c * BLK:(c + 1) * BLK], "w": w[c * BLK:(c + 1) * BLK]}
        for c in range(N_CORES)
    ]
    global _last_nc, _last_in_maps
    _last_nc, _last_in_maps = nc, in_maps
    res = None
    for attempt in range(3):
        try:
            res = run_bass_kernel_spmd(nc, in_maps,
                                       core_ids=list(range(N_CORES)))
            break
        except Exception:
            # transient NRT_EXEC_UNIT_UNRECOVERABLE wedges occur on busy axon
            # terminals; they clear after a pause
            if attempt == 2:
                raise
            import time
            time.sleep(45)

    out = np.empty((BH, S, S), dtype=np.float32)
    for c in range(N_CORES):
        out[c * BLK:(c + 1) * BLK] = res.results[c]["out"]
    return out.reshape(B, H, S, S)



# revision 5
# speedup vs baseline: 1.2442x; 1.2442x over previous
"""Trainium2 Bass kernel for nn_DRAM_MAC_temporal_encoding (polynomial attention).

Math (QK_mul=1):
    out = sum_i coef_i * (x @ (y-OFF)^i) * decay
        = (x * decay) @ P(y-OFF)            # P = Horner cubic, elementwise
so the whole problem is ONE [S,64]@[64,S] matmul per (b,h) head plus the
output write -> memory-bound. The tiny elementwise prep (poly on y,
row-scaling x, transposes, fp16 casts) runs on host; the device does
matmuls + store.

Precision: tolerance is rel_err < 2e-2. fp16 inputs + single fp16 matmul
(fp32 PSUM accumulate) + fp16 output measures 2.5e-4 on the numpy model —
so no hi/lo split and, crucially, the 50 MiB/core fp32 output write
becomes 25 MiB fp16 (host upcasts back to fp32). PSUM->SBUF fp32->fp16
drains rotate across Vector/Scalar/Pool so no single engine bottlenecks.

QK_mul=0: out = sum_i coef_i * ((x*d^i) @ (y-OFF)^i) -> two K=128 chunks
(4 stacked K=64 terms), same kernel with n_chunks=2.

Sharding: 24 (b,h) heads -> 3 per core across 8 cores.
"""

import numpy as np

import concourse.mybir as mybir
import concourse.tile as tile
from concourse import bacc
from concourse.bass_utils import run_bass_kernel_spmd

C = [0.17393044, 0.15653739, 0.14088365, 0.12679529, 5.51975209,
     4.96777688, 4.4709992, -1.44776001, -1.30298401, 46.05483778]
MAX_ORDER = 3
X_MAX = 0.9
OFFSET = 0.45

B, H, S, D = 2, 12, 2048, 64
BH = B * H
N_CORES = 8
BLK = BH // N_CORES  # heads per core

M_TILE = 128   # output rows per matmul (PSUM partitions)
N_TILE = 512   # output cols per matmul (one fp32 PSUM bank)

_NC_CACHE = {}
_last_nc = None
_last_in_maps = None


def _coefs():
    cs = []
    idx = 0
    for i in range(MAX_ORDER + 1):
        n_j = MAX_ORDER - i + 1
        cs.append(sum(C[idx + j] * X_MAX ** j for j in range(n_j)))
        idx += n_j
    return cs  # [c0, c1, c2, c3]


def _build_nc(n_chunks, kp):
    """Device kernel: per core, BLK independent [S,S] fp16 output blocks,
    each output tile = sum over n_chunks K=kp fp16 matmuls."""
    nc = bacc.Bacc(None, target_bir_lowering=False)
    a_d = nc.dram_tensor("a", [BLK, n_chunks, kp, S], mybir.dt.float16,
                         kind="ExternalInput")
    w_d = nc.dram_tensor("w", [BLK, n_chunks, kp, S], mybir.dt.float16,
                         kind="ExternalInput")
    out_d = nc.dram_tensor("out", [BLK, S, S], mybir.dt.float16,
                           kind="ExternalOutput")

    with tile.TileContext(nc) as tc:
        with (
            tc.tile_pool(name="inp", bufs=1) as inp,
            tc.tile_pool(name="ps", bufs=2, space="PSUM") as psp,
            tc.tile_pool(name="outp", bufs=8) as outp,
        ):
            # Prefetch every input tile up front so the steady-state DMA
            # queues carry only output stores.
            a_ts, w_ts = {}, {}
            for blk in range(BLK):
                for c in range(n_chunks):
                    ta = inp.tile([kp, S], mybir.dt.float16,
                                  tag=f"a{blk}_{c}")
                    nc.sync.dma_start(ta[:], a_d[blk, c])
                    a_ts[(blk, c)] = ta
                    tw = inp.tile([kp, S], mybir.dt.float16,
                                  tag=f"w{blk}_{c}")
                    nc.sync.dma_start(tw[:], w_d[blk, c])
                    w_ts[(blk, c)] = tw

            # Pool/GpSimd can't read PSUM on TRN2, so drains split between
            # DVE (~2.26us per 128x2048 copy) and Act (~1.85us); greedy
            # balance by accumulated cost.
            drain_cost = {"v": 0.0, "s": 0.0}
            with nc.allow_low_precision(reason="fp16 out within 2e-2 tol"):
                for blk in range(BLK):
                    for st in range(S // M_TILE):
                        ps = psp.tile([M_TILE, S], mybir.dt.float32, tag="ps")
                        for nt in range(S // N_TILE):
                            for c in range(n_chunks):
                                nc.tensor.matmul(
                                    ps[:, nt * N_TILE:(nt + 1) * N_TILE],
                                    a_ts[(blk, c)][
                                        :, st * M_TILE:(st + 1) * M_TILE],
                                    w_ts[(blk, c)][
                                        :, nt * N_TILE:(nt + 1) * N_TILE],
                                    start=(c == 0),
                                    stop=(c == n_chunks - 1),
                                )
                        ot = outp.tile([M_TILE, S], mybir.dt.float16,
                                       tag="ot")
                        if drain_cost["v"] + 2.26 <= drain_cost["s"] + 1.85:
                            drain_cost["v"] += 2.26
                            nc.vector.tensor_copy(ot[:], ps[:])
                        else:
                            drain_cost["s"] += 1.85
                            nc.scalar.copy(ot[:], ps[:])
                        nc.sync.dma_start(
                            out_d[blk, st * M_TILE:(st + 1) * M_TILE, :],
                            ot[:])
    nc.compile()
    return nc


def _get_nc(n_chunks, kp):
    key = (n_chunks, kp)
    if key not in _NC_CACHE:
        _NC_CACHE[key] = _build_nc(n_chunks, kp)
    return _NC_CACHE[key]


def _prepare(x, y, dm, qk):
    """Host prep -> (a, w) fp16 arrays, both [BH, n_chunks, kp, S]."""
    c0, c1, c2, c3 = _coefs()
    yo = (y - OFFSET).astype(np.float32)  # [B,H,D,S]
    if qk:
        n_chunks, kp = 1, D
        a = np.empty((BH, 1, D, S), dtype=np.float16)
        w = np.empty((BH, 1, D, S), dtype=np.float16)
        a[:, 0] = (x * dm[None, None, :, :]).transpose(0, 1, 3, 2) \
            .reshape(BH, D, S)
        w[:, 0] = (((c3 * yo + c2) * yo + c1) * yo + c0).reshape(BH, D, S)
    else:
        n_chunks, kp = 2, 2 * D
        d = dm[:, 0]
        a = np.empty((BH, 2, 2 * D, S), dtype=np.float16)
        w = np.empty((BH, 2, 2 * D, S), dtype=np.float16)
        xt = x.transpose(0, 1, 3, 2).reshape(BH, D, S)
        di = np.ones_like(d)
        yi = np.ones_like(yo).reshape(BH, D, S)
        yo_r = yo.reshape(BH, D, S)
        for i, ci in enumerate((c0, c1, c2, c3)):
            c, half = divmod(i, 2)
            a[:, c, half * D:(half + 1) * D] = xt * di[None, None, :]
            w[:, c, half * D:(half + 1) * D] = ci * yi
            di = di * d
            yi = yi * yo_r
    return a, w, n_chunks, kp


def kernel(**inputs):
    x = np.asarray(inputs["x"], dtype=np.float32)
    y = np.asarray(inputs["y"], dtype=np.float32)
    dm = np.asarray(inputs["decay_mask"], dtype=np.float32)
    qk = int(np.asarray(inputs["QK_mul"]))

    a, w, n_chunks, kp = _prepare(x, y, dm, qk)
    nc = _get_nc(n_chunks, kp)

    in_maps = [
        {"a": a[c * BLK:(c + 1) * BLK], "w": w[c * BLK:(c + 1) * BLK]}
        for c in range(N_CORES)
    ]
    global _last_nc, _last_in_maps
    _last_nc, _last_in_maps = nc, in_maps

    res = None
    for attempt in range(3):
        try:
            res = run_bass_kernel_spmd(nc, in_maps,
                                       core_ids=list(range(N_CORES)))
            break
        except Exception:
            # transient NRT_EXEC_UNIT_UNRECOVERABLE wedges occur on busy axon
            # terminals; they clear after a pause
            if attempt == 2:
                raise
            import time
            time.sleep(45)

    out = np.empty((BH, S, S), dtype=np.float32)
    for c in range(N_CORES):
        out[c * BLK:(c + 1) * BLK] = res.results[c]["out"]
    return out.reshape(B, H, S, S)


# revision 6
# speedup vs baseline: 1.2448x; 1.0004x over previous
"""Trainium2 Bass kernel for nn_DRAM_MAC_temporal_encoding (polynomial attention).

Math (QK_mul=1):
    out = sum_i coef_i * (x @ (y-OFF)^i) * decay
        = (x * decay) @ P(y-OFF)            # P = Horner cubic, elementwise
so the whole problem is ONE [S,64]@[64,S] matmul per (b,h) head plus the
output write -> memory-bound. The tiny elementwise prep (poly on y,
row-scaling x, transposes, fp16 casts) runs on host; the device does
matmuls + store.

Precision: tolerance is rel_err < 2e-2. fp16 inputs + single fp16 matmul
(fp32 PSUM accumulate) + fp16 output measures 2.5e-4 on the numpy model —
so no hi/lo split and, crucially, the 50 MiB/core fp32 output write
becomes 25 MiB fp16 (host upcasts back to fp32). PSUM->SBUF fp32->fp16
drains rotate across Vector/Scalar/Pool so no single engine bottlenecks.

QK_mul=0: out = sum_i coef_i * ((x*d^i) @ (y-OFF)^i) -> two K=128 chunks
(4 stacked K=64 terms), same kernel with n_chunks=2.

Sharding: 24 (b,h) heads -> 3 per core across 8 cores.
"""

import ml_dtypes
import numpy as np

import concourse.mybir as mybir
import concourse.tile as tile
from concourse import bacc
from concourse.bass_utils import run_bass_kernel_spmd

C = [0.17393044, 0.15653739, 0.14088365, 0.12679529, 5.51975209,
     4.96777688, 4.4709992, -1.44776001, -1.30298401, 46.05483778]
MAX_ORDER = 3
X_MAX = 0.9
OFFSET = 0.45

B, H, S, D = 2, 12, 2048, 64
BH = B * H
N_CORES = 8
BLK = BH // N_CORES  # heads per core

M_TILE = 128   # output rows per matmul (PSUM partitions)
N_TILE = 512   # output cols per matmul (one fp32 PSUM bank)

_NC_CACHE = {}
_last_nc = None
_last_in_maps = None


def _coefs():
    cs = []
    idx = 0
    for i in range(MAX_ORDER + 1):
        n_j = MAX_ORDER - i + 1
        cs.append(sum(C[idx + j] * X_MAX ** j for j in range(n_j)))
        idx += n_j
    return cs  # [c0, c1, c2, c3]


def _build_nc(n_chunks, kp):
    """Device kernel: per core, BLK independent [S,S] fp16 output blocks,
    each output tile = sum over n_chunks K=kp fp16 matmuls."""
    nc = bacc.Bacc(None, target_bir_lowering=False)
    a_d = nc.dram_tensor("a", [BLK, n_chunks, kp, S], mybir.dt.bfloat16,
                         kind="ExternalInput")
    w_d = nc.dram_tensor("w", [BLK, n_chunks, kp, S], mybir.dt.bfloat16,
                         kind="ExternalInput")
    out_d = nc.dram_tensor("out", [BLK, S, S], mybir.dt.float16,
                           kind="ExternalOutput")

    with tile.TileContext(nc) as tc:
        with (
            tc.tile_pool(name="inp", bufs=1) as inp,
            tc.tile_pool(name="ps", bufs=2, space="PSUM") as psp,
            tc.tile_pool(name="outp", bufs=8) as outp,
        ):
            # Prefetch every input tile up front so the steady-state DMA
            # queues carry only output stores.
            a_ts, w_ts = {}, {}
            for blk in range(BLK):
                for c in range(n_chunks):
                    ta = inp.tile([kp, S], mybir.dt.bfloat16,
                                  tag=f"a{blk}_{c}")
                    nc.sync.dma_start(ta[:], a_d[blk, c])
                    a_ts[(blk, c)] = ta
                    tw = inp.tile([kp, S], mybir.dt.bfloat16,
                                  tag=f"w{blk}_{c}")
                    nc.sync.dma_start(tw[:], w_d[blk, c])
                    w_ts[(blk, c)] = tw

            # Pool/GpSimd can't read PSUM on TRN2, so drains split between
            # DVE (~2.26us per 128x2048 copy) and Act (~1.85us); greedy
            # balance by accumulated cost.
            drain_cost = {"v": 0.0, "s": 0.0}
            with nc.allow_low_precision(reason="fp16 out within 2e-2 tol"):
                for blk in range(BLK):
                    for st in range(S // M_TILE):
                        ps = psp.tile([M_TILE, S], mybir.dt.float32, tag="ps")
                        for nt in range(S // N_TILE):
                            for c in range(n_chunks):
                                nc.tensor.matmul(
                                    ps[:, nt * N_TILE:(nt + 1) * N_TILE],
                                    a_ts[(blk, c)][
                                        :, st * M_TILE:(st + 1) * M_TILE],
                                    w_ts[(blk, c)][
                                        :, nt * N_TILE:(nt + 1) * N_TILE],
                                    start=(c == 0),
                                    stop=(c == n_chunks - 1),
                                )
                        ot = outp.tile([M_TILE, S], mybir.dt.float16,
                                       tag="ot")
                        if drain_cost["v"] + 2.26 <= drain_cost["s"] + 1.85:
                            drain_cost["v"] += 2.26
                            nc.vector.tensor_copy(ot[:], ps[:])
                        else:
                            drain_cost["s"] += 1.85
                            nc.scalar.copy(ot[:], ps[:])
                        nc.sync.dma_start(
                            out_d[blk, st * M_TILE:(st + 1) * M_TILE, :],
                            ot[:])
    nc.compile()
    return nc


def _get_nc(n_chunks, kp):
    key = (n_chunks, kp)
    if key not in _NC_CACHE:
        _NC_CACHE[key] = _build_nc(n_chunks, kp)
    return _NC_CACHE[key]


def _prepare(x, y, dm, qk):
    """Host prep -> (a, w) fp16 arrays, both [BH, n_chunks, kp, S]."""
    c0, c1, c2, c3 = _coefs()
    yo = (y - OFFSET).astype(np.float32)  # [B,H,D,S]
    if qk:
        n_chunks, kp = 1, D
        a = np.empty((BH, 1, D, S), dtype=ml_dtypes.bfloat16)
        w = np.empty((BH, 1, D, S), dtype=ml_dtypes.bfloat16)
        a[:, 0] = (x * dm[None, None, :, :]).transpose(0, 1, 3, 2) \
            .reshape(BH, D, S)
        w[:, 0] = (((c3 * yo + c2) * yo + c1) * yo + c0).reshape(BH, D, S)
    else:
        n_chunks, kp = 2, 2 * D
        d = dm[:, 0]
        a = np.empty((BH, 2, 2 * D, S), dtype=ml_dtypes.bfloat16)
        w = np.empty((BH, 2, 2 * D, S), dtype=ml_dtypes.bfloat16)
        xt = x.transpose(0, 1, 3, 2).reshape(BH, D, S)
        di = np.ones_like(d)
        yi = np.ones_like(yo).reshape(BH, D, S)
        yo_r = yo.reshape(BH, D, S)
        for i, ci in enumerate((c0, c1, c2, c3)):
            c, half = divmod(i, 2)
            a[:, c, half * D:(half + 1) * D] = xt * di[None, None, :]
            w[:, c, half * D:(half + 1) * D] = ci * yi
            di = di * d
            yi = yi * yo_r
    return a, w, n_chunks, kp


def kernel(**inputs):
    x = np.asarray(inputs["x"], dtype=np.float32)
    y = np.asarray(inputs["y"], dtype=np.float32)
    dm = np.asarray(inputs["decay_mask"], dtype=np.float32)
    qk = int(np.asarray(inputs["QK_mul"]))

    a, w, n_chunks, kp = _prepare(x, y, dm, qk)
    nc = _get_nc(n_chunks, kp)

    in_maps = [
        {"a": a[c * BLK:(c + 1) * BLK], "w": w[c * BLK:(c + 1) * BLK]}
        for c in range(N_CORES)
    ]
    global _last_nc, _last_in_maps
    _last_nc, _last_in_maps = nc, in_maps

    res = None
    for attempt in range(3):
        try:
            res = run_bass_kernel_spmd(nc, in_maps,
                                       core_ids=list(range(N_CORES)))
            break
        except Exception:
            # transient NRT_EXEC_UNIT_UNRECOVERABLE wedges occur on busy axon
            # terminals; they clear after a pause
            if attempt == 2:
                raise
            import time
            time.sleep(45)

    out = np.empty((BH, S, S), dtype=np.float32)
    for c in range(N_CORES):
        out[c * BLK:(c + 1) * BLK] = res.results[c]["out"]
    return out.reshape(B, H, S, S)


# revision 8
# speedup vs baseline: 1.4806x; 1.1894x over previous
"""Trainium2 Bass kernel for nn_DRAM_MAC_temporal_encoding (polynomial attention).

Math (QK_mul=1):
    out = sum_i coef_i * (x @ (y-OFF)^i) * decay
        = (x * decay) @ P(y-OFF)            # P = Horner cubic, elementwise
so the whole problem is ONE [S,64]@[64,S] matmul per (b,h) head plus the
output write -> memory-bound. The tiny elementwise prep (poly on y,
row-scaling x, transposes, fp16 casts) runs on host; the device does
matmuls + store.

Precision: tolerance is rel_err < 2e-2. fp16 inputs + single fp16 matmul
(fp32 PSUM accumulate) + fp16 output measures 2.5e-4 on the numpy model —
so no hi/lo split and, crucially, the 50 MiB/core fp32 output write
becomes 25 MiB fp16 (host upcasts back to fp32). PSUM->SBUF fp32->fp16
drains rotate across Vector/Scalar/Pool so no single engine bottlenecks.

QK_mul=0: out = sum_i coef_i * ((x*d^i) @ (y-OFF)^i) -> two K=128 chunks
(4 stacked K=64 terms), same kernel with n_chunks=2.

Sharding: 24 (b,h) heads -> 3 per core across 8 cores.
"""

import ml_dtypes
import numpy as np

import concourse.mybir as mybir
import concourse.tile as tile
from concourse import bacc
from concourse.bass_utils import run_bass_kernel_spmd

C = [0.17393044, 0.15653739, 0.14088365, 0.12679529, 5.51975209,
     4.96777688, 4.4709992, -1.44776001, -1.30298401, 46.05483778]
MAX_ORDER = 3
X_MAX = 0.9
OFFSET = 0.45

B, H, S, D = 2, 12, 2048, 64
BH = B * H
N_CORES = 8
BLK = BH // N_CORES  # heads per core

M_TILE = 128   # output rows per matmul (PSUM partitions)
N_TILE = 512   # output cols per matmul (one fp32 PSUM bank)

_NC_CACHE = {}
_last_nc = None
_last_in_maps = None


def _coefs():
    cs = []
    idx = 0
    for i in range(MAX_ORDER + 1):
        n_j = MAX_ORDER - i + 1
        cs.append(sum(C[idx + j] * X_MAX ** j for j in range(n_j)))
        idx += n_j
    return cs  # [c0, c1, c2, c3]


def _build_nc(n_chunks, wk):
    """Device kernel: per core, BLK independent [S,S] fp16 output blocks,
    each output tile = sum over n_chunks K=128 bf16 matmuls.

    K=64 matmuls stream at ~1/3 the K=128 rate on TRN2 HW (630ns vs 233ns
    per [128,512]), so the contraction is always presented as K=128:
    a carries [hi; lo] bf16 rows, and when wk == 64 the w rows are
    replicated in SBUF (two DMAs from the same DRAM region) so one matmul
    computes (a_hi + a_lo) @ w."""
    nc = bacc.Bacc(None, target_bir_lowering=False)
    a_d = nc.dram_tensor("a", [BLK, n_chunks, 128, S], mybir.dt.bfloat16,
                         kind="ExternalInput")
    w_d = nc.dram_tensor("w", [BLK, n_chunks, wk, S], mybir.dt.bfloat16,
                         kind="ExternalInput")
    out_d = nc.dram_tensor("out", [BLK, S, S], mybir.dt.float16,
                           kind="ExternalOutput")

    with tile.TileContext(nc) as tc:
        with (
            tc.tile_pool(name="inp", bufs=1) as inp,
            tc.tile_pool(name="ps", bufs=2, space="PSUM") as psp,
            tc.tile_pool(name="outp", bufs=8) as outp,
        ):
            # Prefetch every input tile up front so the steady-state DMA
            # queues carry only output stores.
            a_ts, w_ts = {}, {}
            for blk in range(BLK):
                for c in range(n_chunks):
                    ta = inp.tile([128, S], mybir.dt.bfloat16,
                                  tag=f"a{blk}_{c}")
                    nc.sync.dma_start(ta[:], a_d[blk, c])
                    a_ts[(blk, c)] = ta
                    tw = inp.tile([128, S], mybir.dt.bfloat16,
                                  tag=f"w{blk}_{c}")
                    if wk == 64:
                        nc.sync.dma_start(tw[:64], w_d[blk, c])
                        nc.sync.dma_start(tw[64:], w_d[blk, c])
                    else:
                        nc.sync.dma_start(tw[:], w_d[blk, c])
                    w_ts[(blk, c)] = tw

            # Pool/GpSimd can't read PSUM on TRN2, so drains split between
            # DVE (~2.26us per 128x2048 copy) and Act (~1.85us); greedy
            # balance by accumulated cost.
            drain_cost = {"v": 0.0, "s": 0.0}
            with nc.allow_low_precision(reason="fp16 out within 2e-2 tol"):
                for blk in range(BLK):
                    for st in range(S // M_TILE):
                        ps = psp.tile([M_TILE, S], mybir.dt.float32, tag="ps")
                        for nt in range(S // N_TILE):
                            for c in range(n_chunks):
                                nc.tensor.matmul(
                                    ps[:, nt * N_TILE:(nt + 1) * N_TILE],
                                    a_ts[(blk, c)][
                                        :, st * M_TILE:(st + 1) * M_TILE],
                                    w_ts[(blk, c)][
                                        :, nt * N_TILE:(nt + 1) * N_TILE],
                                    start=(c == 0),
                                    stop=(c == n_chunks - 1),
                                )
                        ot = outp.tile([M_TILE, S], mybir.dt.float16,
                                       tag="ot")
                        if drain_cost["v"] + 2.26 <= drain_cost["s"] + 1.85:
                            drain_cost["v"] += 2.26
                            nc.vector.tensor_copy(ot[:], ps[:])
                        else:
                            drain_cost["s"] += 1.85
                            nc.scalar.copy(ot[:], ps[:])
                        nc.sync.dma_start(
                            out_d[blk, st * M_TILE:(st + 1) * M_TILE, :],
                            ot[:])
    nc.compile()
    return nc


def _get_nc(n_chunks, wk):
    key = (n_chunks, wk)
    if key not in _NC_CACHE:
        _NC_CACHE[key] = _build_nc(n_chunks, wk)
    return _NC_CACHE[key]


def _hilo(v):
    """f32 -> stacked [hi; lo] bf16 rows along axis -2 (hi+lo ~= v)."""
    hi = v.astype(ml_dtypes.bfloat16)
    lo = (v - hi.astype(np.float32)).astype(ml_dtypes.bfloat16)
    return np.concatenate([hi, lo], axis=-2)


def _prepare(x, y, dm, qk):
    """Host prep -> (a, w) bf16 arrays: a [BH, n_chunks, 128, S],
    w [BH, n_chunks, wk, S]."""
    c0, c1, c2, c3 = _coefs()
    yo = (y - OFFSET).astype(np.float32)  # [B,H,D,S]
    if qk:
        n_chunks, wk = 1, D
        at = np.ascontiguousarray(
            (x * dm[None, None, :, :]).transpose(0, 1, 3, 2)
        ).reshape(BH, D, S)
        a = _hilo(at).reshape(BH, 1, 2 * D, S)
        w = (((c3 * yo + c2) * yo + c1) * yo + c0) \
            .astype(ml_dtypes.bfloat16).reshape(BH, 1, D, S)
    else:
        n_chunks, wk = 2, 2 * D
        d = dm[:, 0]
        a = np.empty((BH, 2, 2 * D, S), dtype=ml_dtypes.bfloat16)
        w = np.empty((BH, 2, 2 * D, S), dtype=ml_dtypes.bfloat16)
        xt = x.transpose(0, 1, 3, 2).reshape(BH, D, S)
        di = np.ones_like(d)
        yi = np.ones_like(yo).reshape(BH, D, S)
        yo_r = yo.reshape(BH, D, S)
        for i, ci in enumerate((c0, c1, c2, c3)):
            c, half = divmod(i, 2)
            a[:, c, half * D:(half + 1) * D] = xt * di[None, None, :]
            w[:, c, half * D:(half + 1) * D] = ci * yi
            di = di * d
            yi = yi * yo_r
    return a, w, n_chunks, wk


def kernel(**inputs):
    x = np.asarray(inputs["x"], dtype=np.float32)
    y = np.asarray(inputs["y"], dtype=np.float32)
    dm = np.asarray(inputs["decay_mask"], dtype=np.float32)
    qk = int(np.asarray(inputs["QK_mul"]))

    a, w, n_chunks, wk = _prepare(x, y, dm, qk)
    nc = _get_nc(n_chunks, wk)

    in_maps = [
        {"a": a[c * BLK:(c + 1) * BLK], "w": w[c * BLK:(c + 1) * BLK]}
        for c in range(N_CORES)
    ]
    global _last_nc, _last_in_maps
    _last_nc, _last_in_maps = nc, in_maps

    res = None
    for attempt in range(3):
        try:
            res = run_bass_kernel_spmd(nc, in_maps,
                                       core_ids=list(range(N_CORES)))
            break
        except Exception:
            # transient NRT_EXEC_UNIT_UNRECOVERABLE wedges occur on busy axon
            # terminals; they clear after a pause
            if attempt == 2:
                raise
            import time
            time.sleep(45)

    out = np.empty((BH, S, S), dtype=np.float32)
    for c in range(N_CORES):
        out[c * BLK:(c + 1) * BLK] = res.results[c]["out"]
    return out.reshape(B, H, S, S)
